# revision 2
# baseline (speedup 1.0000x reference)
"""Trainium2 Bass kernel for the GNN message-passing module (fp8 DoubleRow
fused message+mask matmul with a shared static mask block).

Reference computation (per batch b, one batch per core):
    msg_n = node @ Wn + bn                      (N, MID)
    msg_h = hidden @ Wh + bh                    (N, MID)
    msg_e = edge @ We + be                      (N, N, MID)
    msg_g = graph @ Wg + bg                     (MID,)
    msgs[i,j,:] = msg_n[j] + msg_h[i] + msg_e[i,j] + msg_g
    out_msgs[j,:] = max_i(msgs[i,j,:] * adj[i,j])
    ret = node @ Wo1 + bo1 + hidden @ Wo2 + bo2 + out_msgs @ Wo3 + bo3

Key idea: the HW bottleneck is the PE instruction stream (~0.5us per
matmul instruction regardless of size; moving free dim capped at 512)
and DMA throughput scales with the transfer's PARTITION count (~1.4GB/s
per partition).  fp8 DoubleRow contracts TWO 128-row k-tiles per
instruction, so one matmul computes
    msg_e (We . edge, k-tile0 = 128 e-rows)
  + h_i + adjm      (k-tile1 = one-hot(8) + adjm(1) + 119 zero rows)
-> 128 matmul instructions total, with full-128-partition edge DMAs.

Layouts: ONE [128, 4*8192] fp8 tile: blocks 0..2 = edge ring slots
(full-128-partition DMA per group), block 3 = STATIC mask block shared by
all groups: rows 0..7 one-hot, rows 8..15 = adjm rows of groups 0..7
(one [8,8192] DMA per iteration), rows 16..127 zeros (one Pool memset).
The moving AP picks (slot, mask) blocks via a step-sliced block dim.
comb [128, 32*256] fp8 stationary, per batch m: [We | rows 0..7 =
h_{8m..8m+8} (device), row 8+m//4 = ones (host), zeros elsewhere] --
the ones-row position selects which adjm row is active for group m//4.
PSUM 2x[128,2048]; drains 2048-wide: DVE 1 direct + ACT 3 evac (DVE
bf16 2x maxes).
PSUM drains: DVE maxes 2/8 [128,1024] tiles from PSUM, ACT evacuates
6/8 to bf16 (DVE bf16 2x maxes).  No accumulator memsets (group 0
copies).  msg_n added after the max; cvec restores the reference's
"0 candidate" masked-max semantics.
"""

from contextlib import ExitStack

import numpy as np

B, N, D, E, G, MID, OUT = 8, 256, 128, 128, 128, 128, 128
NCORES = 8
BIG = 1.0e30
A8 = 240.0         # fp8e4m3 (ml_dtypes float8_e4m3) additive mask magnitude
GI = 32            # sender rows (i values) per edge group / DMA
NG = N // GI       # 8 groups
NBATCH = 4         # batches per group (8 senders each)
NT = N // 128      # number of 128-row tiles along N

_WNAMES = ["Wn", "Wh", "We", "Wg", "Wo1", "Wo2", "Wo3"]
_BNAMES = ["bn", "bh", "be", "bg", "bo1", "bo2", "bo3"]

_CACHE = {}

# batch (0..3 within a group) -> drain path
_DRAIN = {0: "dve", 1: "actd", 2: "actd", 3: "actd"}


def _ensure_path():
    try:
        import concourse.bass  # noqa: F401
    except ImportError:
        import sys

        for p in ("/opt/trn_rl_repo", "/root/.axon_site/_ro/trn_rl_repo"):
            if p not in sys.path:
                sys.path.insert(0, p)
        import concourse.bass  # noqa: F401


def _dedup_ldweights(m):
    """Remove back-to-back duplicate PE LDWEIGHTS (same stationary)."""
    n = 0
    for fn in m.functions:
        for blk in fn.blocks:
            last = None
            doomed = []
            for inst in list(blk.instructions):
                if str(getattr(inst, "engine", "")) != "EngineType.PE":
                    continue
                nm = type(inst).__name__
                if nm == "InstLdweights":
                    si = inst.sync_info
                    clean = si is None or (not si.on_wait and not si.on_update)
                    sig = (
                        repr(inst.ins[0]),
                        str(inst.perf_mode),
                        str(inst.tile_position),
                        str(inst.is_transpose),
                    )
                    if sig == last and clean:
                        doomed.append(inst)
                    else:
                        last = sig
                elif nm == "InstMatmult":
                    continue
                else:
                    last = None
            for inst in doomed:
                blk.instructions.remove(inst)
            n += len(doomed)
    return n


def _kernel_body(ctx, tc, aps, rep=0, edge_groups=None, ablate=()):
    import concourse.bass as bass  # noqa: F401
    from concourse import masks, mybir

    nc = tc.nc
    f32 = mybir.dt.float32
    f32r = mybir.dt.float32r
    bf16 = mybir.dt.bfloat16
    fp8 = mybir.dt.float8e4
    Alu = mybir.AluOpType
    DR = mybir.MatmulPerfMode.DoubleRow

    edge = aps["edge"]
    out = aps["out"]

    const = ctx.enter_context(tc.tile_pool(name="const", bufs=1))
    opool = ctx.enter_context(tc.tile_pool(name="op", bufs=2, space="PSUM"))
    ps_pool = opool
    scratch = ctx.enter_context(tc.tile_pool(name="scratch", bufs=1))
    stpool = ctx.enter_context(tc.tile_pool(name="st", bufs=4))

    # ---- constants -------------------------------------------------------
    ident = const.tile([128, 128], f32)
    masks.make_identity(nc, ident[:])

    ones_f = scratch.tile([1, 256], f32)
    nc.vector.memset(ones_f[:], 1.0)
    ones_row = const.tile([1, 256], f32r)
    nc.vector.tensor_copy(ones_row[:], ones_f[:])
    ones_1c = const.tile([1, 128], f32r)
    nc.vector.tensor_copy(ones_1c[:], ones_f[:, 0:128])
    ones_11 = const.tile([1, 1], f32r)
    nc.vector.tensor_copy(ones_11[:], ones_f[:, 0:1])
    ones_colf = scratch.tile([128, 1], f32)
    nc.vector.memset(ones_colf[:], 1.0)
    ones_col = const.tile([128, 1], bf16)
    nc.vector.tensor_copy(ones_col[:], ones_colf[:])

    # wpack: 7 weight matrices + node/hid natural tiles + graph col, 1 DMA
    WP = 7 * 128
    wpack_sb = scratch.tile([128, WP + 2 * NT * 128 + 1], f32)
    nc.sync.dma_start(wpack_sb[:], aps["wpack"])
    bpack_sb = scratch.tile([1, 2 * 128], f32)
    nc.sync.dma_start(bpack_sb[:], aps["bpack"].rearrange("(o k) -> o k", o=1))

    W_sb = {}
    for i, w in enumerate(_WNAMES):
        W_sb[w] = const.tile([128, 128], f32r, name=f"r{rep}_W_{w}", tag=f"W_{w}")
        nc.vector.tensor_copy(W_sb[w][:], wpack_sb[:, i * 128 : (i + 1) * 128])
    B_sb = {}
    for i, b in enumerate(["bsum", "bosum"]):
        B_sb[b] = const.tile([1, 128], f32r, name=f"r{rep}_B_{b}", tag=f"B_{b}")
        nc.vector.tensor_copy(B_sb[b][:], bpack_sb[:, i * 128 : (i + 1) * 128])

    node_off = WP
    hid_off = WP + NT * 128
    graph_col = const.tile([128, 1], f32r)
    nc.vector.tensor_copy(graph_col[:], wpack_sb[:, WP + 2 * NT * 128 :])

    # ---- per-batch precompute -------------------------------------------
    nodeT = const.tile([128, 256], f32r)
    hidT = const.tile([128, 256], f32r)
    for off, T in ((node_off, nodeT), (hid_off, hidT)):
        ps = ps_pool.tile([128, 256], f32, tag="op")
        for t in range(NT):
            nc.tensor.transpose(
                ps[:, t * 128 : (t + 1) * 128],
                wpack_sb[:, off + t * 128 : off + (t + 1) * 128],
                ident[:],
            )
        nc.scalar.copy(T[:], ps[:])

    # r0 = graph @ Wg + (bn + bh + be + bg)  (bias sum packed on host)
    ps_r0 = ps_pool.tile([128, 256], f32, tag="op")
    nc.tensor.matmul(
        ps_r0[0:1, 0:128], graph_col[:], W_sb["Wg"][:], start=True, stop=False
    )
    nc.tensor.matmul(
        ps_r0[0:1, 0:128], ones_11[:], B_sb["bsum"][:], start=False, stop=True
    )
    r0 = const.tile([1, 128], f32r)
    nc.scalar.copy(r0[:], ps_r0[0:1, 0:128])

    # H[i, c] = hidden @ Wh + r0 -> fp8, h_dram row i
    h_dram = aps["h_scratch"]
    ps_h = ps_pool.tile([128, 256], f32, tag="op")
    for t in range(NT):
        nc.tensor.matmul(
            ps_h[:, t * 128 : (t + 1) * 128],
            hidT[:, t * 128 : (t + 1) * 128],
            W_sb["Wh"][:],
            start=True, stop=False,
        )
        nc.tensor.matmul(
            ps_h[:, t * 128 : (t + 1) * 128],
            ones_1c[:], r0[:],
            start=False, stop=True,
        )
    H_f8 = scratch.tile([128, 256], fp8)
    nc.scalar.copy(H_f8[:], ps_h[:])
    nc.sync.dma_start(
        h_dram.bitcast(f32).rearrange("(t p c) -> p t c", t=NT, p=128),
        H_f8[:].bitcast(f32).rearrange("p (t c) -> p t c", t=NT),
    )

    # comb stationary [128, NG*256] fp8: host static part + device h rows
    comb = const.tile([128, NG * 256], fp8)
    nc.sync.dma_start(comb[:].bitcast(f32), aps["combstat"].bitcast(f32))
    nc.sync.dma_start(
        comb[0:32, :].bitcast(f32).rearrange(
            "r (m c) -> r m c", m=NG, c=64
        )[:, :, 32:64],
        h_dram.bitcast(f32).rearrange("(m r c) -> r m c", r=GI, c=32),
    )

    # msg_nT[c, j] = (node @ Wn).T
    ps_mn = ps_pool.tile([128, 256], f32, tag="op")
    nc.tensor.matmul(ps_mn[:], W_sb["Wn"][:], nodeT[:], start=True, stop=True)
    msg_nT = const.tile([128, 256], f32)
    nc.scalar.copy(msg_nT[:], ps_mn[:])

    # cvec[j]: host-computed (-BIG if column fully kept, else 0)
    cvec_f = scratch.tile([1, 256], f32)
    nc.sync.dma_start(cvec_f[:], aps["cvec_in"].rearrange("(o k) -> o k", o=1))
    cvec = const.tile([1, 256], f32r)
    nc.vector.tensor_copy(cvec[:], cvec_f[:])

    # ---- big tile: 3 edge ring slots + 1 shared static mask block --------
    BW = GI * 256
    bigt = const.tile([128, 4 * BW], fp8, name=f"r{rep}_bigt")
    nc.gpsimd.memset(bigt[:, 3 * BW :], 0.0)
    nc.gpsimd.dma_start(
        bigt[0:40, 3 * BW :].bitcast(f32), aps["maskrows"].bitcast(f32)
    )
    bigv = bigt[:].rearrange("p (b c) -> p b c", b=4)

    # accumulators (bf16; group 0 drains use copies, so no memsets)
    accD = const.tile([128, 2048], bf16, name=f"r{rep}_accD", tag="accD")
    accA = const.tile([128, 2048], bf16, name=f"r{rep}_accA", tag="accA")
    accs = {"dve": accD, "actd": accA}
    first = {"dve": True, "actd": True}

    # ---- main loop over sender-row groups -------------------------------

    def stage_a(g):
        """Load edge group g into its ring slot (one 128-partition DMA)."""
        gsrc = g if edge_groups is None else (g % edge_groups)
        sl = g % 3
        if "noet" not in ablate:
            if "split" in ablate:
                h = BW // 2
                nc.sync.dma_start(
                    bigt[:, sl * BW : sl * BW + h].bitcast(f32),
                    edge[gsrc][:, 0:h].bitcast(f32),
                )
                nc.scalar.dma_start(
                    bigt[:, sl * BW + h : (sl + 1) * BW].bitcast(f32),
                    edge[gsrc][:, h:BW].bitcast(f32),
                )
            else:
                eng = nc.sync if (g % 2 == 0 or "oneq" in ablate) else nc.scalar
                eng.dma_start(
                    bigt[:, sl * BW : (sl + 1) * BW].bitcast(f32),
                    edge[gsrc].bitcast(f32),
                )
        return sl

    no_we = "we" in ablate
    no_dve = "dve" in ablate

    def stage_b(g, sl):
        """Fused DoubleRow matmuls (512 cols each) + 2048-wide drains."""
        if no_we:
            return
        sv = bigv[:, sl : 4 : (3 - sl) if sl < 3 else 1, :]
        lhsT = comb[:, g * 256 : (g + 1) * 256].rearrange(
            "p (u w) -> p u w", u=2
        )
        for b in range(NBATCH):
            op = opool.tile(
                [128, 2048], f32, tag="op", name=f"r{rep}_op{g}_{b}"
            )
            c0 = b * 2048
            for q in range(4):
                nc.tensor.matmul(
                    op[:, q * 512 : (q + 1) * 512],
                    lhsT,
                    sv[:, :, c0 + q * 512 : c0 + (q + 1) * 512],
                    start=True, stop=True,
                    perf_mode=DR,
                )
            if no_dve:
                continue
            eng = _DRAIN[b]
            acc = accs[eng]
            if eng == "dve":
                if first[eng]:
                    nc.vector.tensor_copy(acc[:], op[:])
                else:
                    nc.vector.tensor_tensor(acc[:], op[:], acc[:], Alu.max)
            else:  # actd: ACT evacuates to bf16, DVE maxes from SBUF
                st = stpool.tile(
                    [128, 2048], bf16, tag="st", name=f"r{rep}_st{g}_{b}"
                )
                nc.scalar.copy(st[:], op[:])
                if first[eng]:
                    nc.vector.tensor_copy(acc[:], st[:])
                else:
                    nc.vector.tensor_tensor(acc[:], st[:], acc[:], Alu.max)
            first[eng] = False

    if "loop" not in ablate:
        depth = 2 if "pf2" in ablate else 1
        pend = []
        for g in range(NG):
            sl = stage_a(g)
            pend.append((g, sl))
            if len(pend) > depth:
                stage_b(*pend.pop(0))
        for args in pend:
            stage_b(*args)

    # ---- finalize --------------------------------------------------------
    for eng, acc in accs.items():
        if first[eng]:  # ablation runs that never touched this accumulator
            nc.vector.memset(acc[:], -3.0e38)
    r01 = const.tile([128, 2048], bf16)
    nc.vector.tensor_tensor(r01[:], accD[:], accA[:], Alu.max)
    rq = const.tile([128, 1024], bf16)
    nc.vector.tensor_tensor(rq[:], r01[:, 0:1024], r01[:, 1024:2048], Alu.max)
    rh = const.tile([128, 512], bf16)
    nc.vector.tensor_tensor(rh[:], rq[:, 0:512], rq[:, 512:1024], Alu.max)
    acc256 = const.tile([128, 256], bf16)
    nc.vector.tensor_tensor(acc256[:], rh[:, 0:256], rh[:, 256:512], Alu.max)

    ps_cv = ps_pool.tile([128, 256], f32, tag="op")
    nc.tensor.matmul(ps_cv[:], ones_1c[:], cvec[:], start=True, stop=True)
    msgsT = const.tile([128, 256], f32)
    nc.vector.tensor_tensor(msgsT[:], acc256[:], msg_nT[:], Alu.add)
    resT = const.tile([128, 256], f32r)
    nc.vector.tensor_tensor(resT[:], msgsT[:], ps_cv[:], Alu.max)

    # ret_T (o, n)
    ps_ret = ps_pool.tile([128, 256], f32, tag="op")
    nc.tensor.matmul(ps_ret[:], W_sb["Wo1"][:], nodeT[:], start=True, stop=False)
    nc.tensor.matmul(ps_ret[:], W_sb["Wo2"][:], hidT[:], start=False, stop=False)
    nc.tensor.matmul(ps_ret[:], W_sb["Wo3"][:], resT[:], start=False, stop=False)
    nc.tensor.matmul(
        ps_ret[:], B_sb["bosum"][:], ones_row[:], start=False, stop=True
    )
    retT = const.tile([128, 256], f32)
    nc.scalar.copy(retT[:], ps_ret[:])

    ps_out = ps_pool.tile([128, 256], f32, tag="op")
    for t in range(NT):
        nc.tensor.transpose(
            ps_out[:, t * 128 : (t + 1) * 128],
            retT[:, t * 128 : (t + 1) * 128],
            ident[:],
        )
    out_sb = const.tile([128, 256], f32)
    nc.scalar.copy(out_sb[:], ps_out[:])
    nc.sync.dma_start(
        out.rearrange("(t p) o -> p t o", p=128),
        out_sb[:].rearrange("p (t o) -> p t o", t=NT),
    )


def build_nc(repeat=1, edge_groups=None, loop_n=1, ablate=()):
    """Build the (single-core SPMD) Bass program; returns nc."""
    _ensure_path()
    import concourse.tile as tile
    from concourse import bacc, mybir

    f32 = mybir.dt.float32
    i32 = mybir.dt.int32
    fp8 = mybir.dt.float8e4
    bf16 = mybir.dt.bfloat16

    nc = bacc.Bacc(
        "TRN2", target_bir_lowering=False, debug=False, num_devices=NCORES
    )
    n_groups = NG if edge_groups is None else edge_groups
    WPCOLS = 7 * 128 + 2 * (N // 128) * 128 + 1
    aps = {
        "edge": nc.dram_tensor(
            "edge", [n_groups, E, GI * N], fp8, kind="ExternalInput"
        ).ap(),
        "wpack": nc.dram_tensor(
            "wpack", [128, WPCOLS], f32, kind="ExternalInput"
        ).ap(),
        "bpack": nc.dram_tensor("bpack", [2 * 128], f32, kind="ExternalInput").ap(),
        "cvec_in": nc.dram_tensor("cvec_in", [256], f32, kind="ExternalInput").ap(),
        "combstat": nc.dram_tensor(
            "combstat", [128, NG * 256], fp8, kind="ExternalInput"
        ).ap(),
        "maskrows": nc.dram_tensor(
            "maskrows", [40, GI * 256], fp8, kind="ExternalInput"
        ).ap(),
        "out": nc.dram_tensor("out", [N, OUT], f32, kind="ExternalOutput").ap(),
    }
    aps["h_scratch"] = nc.dram_tensor("h_scratch", [N * MID], fp8).ap()

    with tile.TileContext(nc) as tc:
        if loop_n > 1:
            with tc.For_i(0, loop_n, 1):
                with ExitStack() as ctx:
                    _kernel_body(
                        ctx, tc, aps, rep=0, edge_groups=edge_groups, ablate=ablate
                    )
        else:
            for rep in range(repeat):
                with ExitStack() as ctx:
                    _kernel_body(
                        ctx, tc, aps, rep=rep, edge_groups=edge_groups, ablate=ablate
                    )
    _dedup_ldweights(nc.m)
    nc.compile()
    return nc


def _get_nc():
    if "nc" not in _CACHE:
        _CACHE["nc"] = build_nc()
    return _CACHE["nc"]


def _f8(x):
    import ml_dtypes

    return np.asarray(x, np.float32).astype(ml_dtypes.float8_e4m3)


def _pack_edge(e):
    """(N, N, E) f32 -> (NG, E, GI*N) fp8 in (g, e, a, j) order."""
    x = np.asarray(e, np.float32).transpose(0, 2, 1)          # (i, e, j)
    x = x.reshape(NG, GI, E, N).transpose(0, 2, 1, 3)         # (g, e, a, j)
    return np.ascontiguousarray(_f8(x.reshape(NG, E, GI * N)))


def _maskrows_np(adj_mat):
    """[40, 8192] fp8: rows 0..31 one-hot, rows 32..39 = adjm rows of
    groups 0..7 ((adj-1)*A8 at (sender 32g+k, j) -> row 32+g col k*256+j)."""
    mr = np.zeros((40, GI * 256), np.float32)
    cols = np.arange(GI * 256)
    mr[cols // 256, cols] = 1.0
    am = (np.asarray(adj_mat, np.float32) - 1.0) * A8      # (i, j)
    mr[32:40] = am.reshape(NG, GI * 256)
    return _f8(mr)


def _combstat_np(We):
    """[128, NG*256] fp8: per group g: [We | h slots(0) rows 0..32,
    ones at row 32+g, zeros elsewhere]."""
    cs = np.zeros((128, NG * 256), np.float32)
    Wef = np.asarray(We, np.float32)
    for g in range(NG):
        cs[:, g * 256 : g * 256 + 128] = Wef
        cs[32 + g, g * 256 + 128 : (g + 1) * 256] = 1.0
    return _f8(cs)


def _nat_tiles(x):
    return np.ascontiguousarray(
        np.asarray(x, np.float32)
        .reshape(NT, 128, 128)
        .transpose(1, 0, 2)
        .reshape(128, NT * 128)
    )


def make_in_maps(**inputs):
    """Shard full inputs into per-core input maps (batch-parallel)."""
    cs = _combstat_np(inputs["We"])
    wcommon = np.concatenate(
        [np.ascontiguousarray(inputs[w], np.float32) for w in _WNAMES], axis=1
    )
    bsum = (np.asarray(inputs["bn"]) + inputs["bh"] + inputs["be"]
            + inputs["bg"]).astype(np.float32)
    bosum = (np.asarray(inputs["bo1"]) + inputs["bo2"]
             + inputs["bo3"]).astype(np.float32)
    bpack = np.concatenate([bsum, bosum])
    in_maps = []
    for c in range(NCORES):
        wpack = np.concatenate(
            [
                wcommon,
                _nat_tiles(inputs["node_fts"][c]),
                _nat_tiles(inputs["hidden"][c]),
                np.asarray(inputs["graph_fts"][c], np.float32).reshape(128, 1),
            ],
            axis=1,
        )
        am = np.asarray(inputs["adj_mat"][c], np.float32)
        cvec_in = np.where(am.min(axis=0) > 0.5, -BIG, 0.0).astype(np.float32)
        m = {
            "edge": _pack_edge(inputs["edge_fts"][c]),
            "wpack": np.ascontiguousarray(wpack, np.float32),
            "bpack": bpack,
            "cvec_in": cvec_in,
            "combstat": cs,
            "maskrows": _maskrows_np(inputs["adj_mat"][c]),
        }
        in_maps.append(m)
    return in_maps


def kernel(**inputs) -> np.ndarray:
    """Full-input entry point: shards over 8 cores, returns (B, N, OUT)."""
    _ensure_path()
    from concourse import bass_utils

    nc = _get_nc()
    in_maps = make_in_maps(**inputs)
    res = bass_utils.run_bass_kernel_spmd(nc, in_maps, core_ids=list(range(NCORES)))
    outs = [res.results[c]["out"] for c in range(NCORES)]
    return np.stack(outs, axis=0).astype(np.float32)


if __name__ == "__main__":
    rng = np.random.default_rng(0)
    inputs = {
        "node_fts": rng.normal(size=(B, N, D)).astype(np.float32),
        "edge_fts": rng.normal(size=(B, N, N, E)).astype(np.float32),
        "graph_fts": rng.normal(size=(B, G)).astype(np.float32),
        "adj_mat": rng.integers(0, 2, size=(B, N, N)).astype(np.int32),
        "hidden": rng.normal(size=(B, N, D)).astype(np.float32),
    }
    s = 0.02
    for w in _WNAMES:
        inputs[w] = (s * rng.normal(size=(128, 128))).astype(np.float32)
    for b in _BNAMES:
        inputs[b] = np.zeros(128, np.float32)
    out = kernel(**inputs)
    print(out.shape, out.dtype)


# revision 3
# speedup vs baseline: 1.0188x; 1.0188x over previous
"""Trainium2 Bass kernel for the GNN message-passing module (fp8 DoubleRow
fused message+mask matmul with a shared static mask block).

Reference computation (per batch b, one batch per core):
    msg_n = node @ Wn + bn                      (N, MID)
    msg_h = hidden @ Wh + bh                    (N, MID)
    msg_e = edge @ We + be                      (N, N, MID)
    msg_g = graph @ Wg + bg                     (MID,)
    msgs[i,j,:] = msg_n[j] + msg_h[i] + msg_e[i,j] + msg_g
    out_msgs[j,:] = max_i(msgs[i,j,:] * adj[i,j])
    ret = node @ Wo1 + bo1 + hidden @ Wo2 + bo2 + out_msgs @ Wo3 + bo3

Key idea: the HW bottleneck is the PE instruction stream (~0.5us per
matmul instruction regardless of size; moving free dim capped at 512)
and DMA throughput scales with the transfer's PARTITION count (~1.4GB/s
per partition).  fp8 DoubleRow contracts TWO 128-row k-tiles per
instruction, so one matmul computes
    msg_e (We . edge, k-tile0 = 128 e-rows)
  + h_i + adjm      (k-tile1 = one-hot(8) + adjm(1) + 119 zero rows)
-> 128 matmul instructions total, with full-128-partition edge DMAs.

Layouts: ONE [128, 4*8192] fp8 tile: blocks 0..2 = edge ring slots
(full-128-partition DMA per group), block 3 = STATIC mask block shared by
all groups: rows 0..7 one-hot, rows 8..15 = adjm rows of groups 0..7
(one [8,8192] DMA per iteration), rows 16..127 zeros (one Pool memset).
The moving AP picks (slot, mask) blocks via a step-sliced block dim.
comb [128, 32*256] fp8 stationary, per batch m: [We | rows 0..7 =
h_{8m..8m+8} (device), row 8+m//4 = ones (host), zeros elsewhere] --
the ones-row position selects which adjm row is active for group m//4.
PSUM 2x[128,2048]; drains 2048-wide: DVE 1 direct + ACT 3 evac (DVE
bf16 2x maxes).
PSUM drains: DVE maxes 2/8 [128,1024] tiles from PSUM, ACT evacuates
6/8 to bf16 (DVE bf16 2x maxes).  No accumulator memsets (group 0
copies).  msg_n added after the max; cvec restores the reference's
"0 candidate" masked-max semantics.
"""

from contextlib import ExitStack

import numpy as np

B, N, D, E, G, MID, OUT = 8, 256, 128, 128, 128, 128, 128
NCORES = 8
BIG = 1.0e30
A8 = 240.0         # fp8e4m3 (ml_dtypes float8_e4m3) additive mask magnitude
GI = 32            # sender rows (i values) per edge group / DMA
NG = N // GI       # 8 groups
NBATCH = 4         # batches per group (8 senders each)
NT = N // 128      # number of 128-row tiles along N

_WNAMES = ["Wn", "Wh", "We", "Wg", "Wo1", "Wo2", "Wo3"]
_BNAMES = ["bn", "bh", "be", "bg", "bo1", "bo2", "bo3"]

_CACHE = {}

# batch (0..3 within a group) -> drain path
_DRAIN = {0: "dve", 1: "actd", 2: "actd", 3: "actd"}


def _ensure_path():
    try:
        import concourse.bass  # noqa: F401
    except ImportError:
        import sys

        for p in ("/opt/trn_rl_repo", "/root/.axon_site/_ro/trn_rl_repo"):
            if p not in sys.path:
                sys.path.insert(0, p)
        import concourse.bass  # noqa: F401


def _dedup_ldweights(m):
    """Remove back-to-back duplicate PE LDWEIGHTS (same stationary)."""
    n = 0
    for fn in m.functions:
        for blk in fn.blocks:
            last = None
            doomed = []
            for inst in list(blk.instructions):
                if str(getattr(inst, "engine", "")) != "EngineType.PE":
                    continue
                nm = type(inst).__name__
                if nm == "InstLdweights":
                    si = inst.sync_info
                    clean = si is None or (not si.on_wait and not si.on_update)
                    sig = (
                        repr(inst.ins[0]),
                        str(inst.perf_mode),
                        str(inst.tile_position),
                        str(inst.is_transpose),
                    )
                    if sig == last and clean:
                        doomed.append(inst)
                    else:
                        last = sig
                elif nm == "InstMatmult":
                    continue
                else:
                    last = None
            for inst in doomed:
                blk.instructions.remove(inst)
            n += len(doomed)
    return n


def _kernel_body(ctx, tc, aps, rep=0, edge_groups=None, ablate=()):
    import concourse.bass as bass  # noqa: F401
    from concourse import masks, mybir

    nc = tc.nc
    f32 = mybir.dt.float32
    f32r = mybir.dt.float32r
    bf16 = mybir.dt.bfloat16
    fp8 = mybir.dt.float8e4
    Alu = mybir.AluOpType
    DR = mybir.MatmulPerfMode.DoubleRow

    edge = aps["edge"]
    out = aps["out"]

    const = ctx.enter_context(tc.tile_pool(name="const", bufs=1))
    opool = ctx.enter_context(tc.tile_pool(name="op", bufs=2, space="PSUM"))
    ps_pool = opool
    scratch = ctx.enter_context(tc.tile_pool(name="scratch", bufs=1))
    stpool = ctx.enter_context(tc.tile_pool(name="st", bufs=4))

    # ---- constants -------------------------------------------------------
    ident = const.tile([128, 128], f32)
    masks.make_identity(nc, ident[:])

    ones_f = scratch.tile([1, 256], f32)
    nc.vector.memset(ones_f[:], 1.0)
    ones_row = const.tile([1, 256], f32r)
    nc.vector.tensor_copy(ones_row[:], ones_f[:])
    ones_1c = const.tile([1, 128], f32r)
    nc.vector.tensor_copy(ones_1c[:], ones_f[:, 0:128])
    ones_11 = const.tile([1, 1], f32r)
    nc.vector.tensor_copy(ones_11[:], ones_f[:, 0:1])
    ones_colf = scratch.tile([128, 1], f32)
    nc.vector.memset(ones_colf[:], 1.0)
    ones_col = const.tile([128, 1], bf16)
    nc.vector.tensor_copy(ones_col[:], ones_colf[:])

    # wpack: 7 weight matrices + node/hid natural tiles + graph col, 1 DMA
    WP = 7 * 128
    wpack_sb = scratch.tile([128, WP + 2 * NT * 128 + 1], f32)
    nc.scalar.dma_start(wpack_sb[:, WP:], aps["wpack"][:, WP:])
    nc.sync.dma_start(wpack_sb[:, 0:WP], aps["wpack"][:, 0:WP])
    bpack_sb = scratch.tile([1, 2 * 128], f32)
    nc.sync.dma_start(bpack_sb[:], aps["bpack"].rearrange("(o k) -> o k", o=1))

    W_sb = {}
    for i, w in enumerate(_WNAMES):
        W_sb[w] = const.tile([128, 128], f32r, name=f"r{rep}_W_{w}", tag=f"W_{w}")
        nc.vector.tensor_copy(W_sb[w][:], wpack_sb[:, i * 128 : (i + 1) * 128])
    B_sb = {}
    for i, b in enumerate(["bsum", "bosum"]):
        B_sb[b] = const.tile([1, 128], f32r, name=f"r{rep}_B_{b}", tag=f"B_{b}")
        nc.vector.tensor_copy(B_sb[b][:], bpack_sb[:, i * 128 : (i + 1) * 128])

    node_off = WP
    hid_off = WP + NT * 128
    graph_col = const.tile([128, 1], f32r)
    nc.vector.tensor_copy(graph_col[:], wpack_sb[:, WP + 2 * NT * 128 :])

    # ---- per-batch precompute -------------------------------------------
    nodeT = const.tile([128, 256], f32r)
    hidT = const.tile([128, 256], f32r)
    for off, T in ((node_off, nodeT), (hid_off, hidT)):
        ps = ps_pool.tile([128, 256], f32, tag="op")
        for t in range(NT):
            nc.tensor.transpose(
                ps[:, t * 128 : (t + 1) * 128],
                wpack_sb[:, off + t * 128 : off + (t + 1) * 128],
                ident[:],
            )
        nc.scalar.copy(T[:], ps[:])

    # r0 = graph @ Wg + (bn + bh + be + bg)  (bias sum packed on host)
    ps_r0 = ps_pool.tile([128, 256], f32, tag="op")
    nc.tensor.matmul(
        ps_r0[0:1, 0:128], graph_col[:], W_sb["Wg"][:], start=True, stop=False
    )
    nc.tensor.matmul(
        ps_r0[0:1, 0:128], ones_11[:], B_sb["bsum"][:], start=False, stop=True
    )
    r0 = const.tile([1, 128], f32r)
    nc.scalar.copy(r0[:], ps_r0[0:1, 0:128])

    # H[i, c] = hidden @ Wh + r0 -> fp8, h_dram row i
    h_dram = aps["h_scratch"]
    ps_h = ps_pool.tile([128, 256], f32, tag="op")
    for t in range(NT):
        nc.tensor.matmul(
            ps_h[:, t * 128 : (t + 1) * 128],
            hidT[:, t * 128 : (t + 1) * 128],
            W_sb["Wh"][:],
            start=True, stop=False,
        )
        nc.tensor.matmul(
            ps_h[:, t * 128 : (t + 1) * 128],
            ones_1c[:], r0[:],
            start=False, stop=True,
        )
    H_f8 = scratch.tile([128, 256], fp8)
    nc.scalar.copy(H_f8[:], ps_h[:])
    nc.sync.dma_start(
        h_dram.bitcast(f32).rearrange("(t p c) -> p t c", t=NT, p=128),
        H_f8[:].bitcast(f32).rearrange("p (t c) -> p t c", t=NT),
    )

    # comb stationary [128, NG*256] fp8: host static part + device h rows
    comb = const.tile([128, NG * 256], fp8)
    nc.sync.dma_start(comb[:].bitcast(f32), aps["combstat"].bitcast(f32))
    nc.sync.dma_start(
        comb[0:32, :].bitcast(f32).rearrange(
            "r (m c) -> r m c", m=NG, c=64
        )[:, :, 32:64],
        h_dram.bitcast(f32).rearrange("(m r c) -> r m c", r=GI, c=32),
    )

    # msg_nT[c, j] = (node @ Wn).T
    ps_mn = ps_pool.tile([128, 256], f32, tag="op")
    nc.tensor.matmul(ps_mn[:], W_sb["Wn"][:], nodeT[:], start=True, stop=True)
    msg_nT = const.tile([128, 256], f32)
    nc.scalar.copy(msg_nT[:], ps_mn[:])

    # cvec[j]: host-computed (-BIG if column fully kept, else 0)
    cvec_f = scratch.tile([1, 256], f32)
    nc.sync.dma_start(cvec_f[:], aps["cvec_in"].rearrange("(o k) -> o k", o=1))
    cvec = const.tile([1, 256], f32r)
    nc.vector.tensor_copy(cvec[:], cvec_f[:])

    # ---- big tile: 3 edge ring slots + 1 shared static mask block --------
    BW = GI * 256
    bigt = const.tile([128, 4 * BW], fp8, name=f"r{rep}_bigt")
    nc.gpsimd.memset(bigt[:, 3 * BW :], 0.0)
    nc.gpsimd.dma_start(
        bigt[0:40, 3 * BW :].bitcast(f32), aps["maskrows"].bitcast(f32)
    )
    bigv = bigt[:].rearrange("p (b c) -> p b c", b=4)

    # accumulators (bf16; group 0 drains use copies, so no memsets)
    accD = const.tile([128, 2048], bf16, name=f"r{rep}_accD", tag="accD")
    accA = const.tile([128, 2048], bf16, name=f"r{rep}_accA", tag="accA")
    accs = {"dve": accD, "actd": accA}
    first = {"dve": True, "actd": True}

    # ---- main loop over sender-row groups -------------------------------

    def stage_a(g):
        """Load edge group g into its ring slot (one 128-partition DMA)."""
        gsrc = g if edge_groups is None else (g % edge_groups)
        sl = g % 3
        if "noet" not in ablate:
            if "split" in ablate:
                h = BW // 2
                nc.sync.dma_start(
                    bigt[:, sl * BW : sl * BW + h].bitcast(f32),
                    edge[gsrc][:, 0:h].bitcast(f32),
                )
                nc.scalar.dma_start(
                    bigt[:, sl * BW + h : (sl + 1) * BW].bitcast(f32),
                    edge[gsrc][:, h:BW].bitcast(f32),
                )
            else:
                if "oneq" in ablate:
                    eng = nc.sync
                elif "q3" in ablate:
                    eng = (nc.sync, nc.scalar, nc.gpsimd)[g % 3]
                else:
                    eng = nc.sync if g % 2 == 0 else nc.scalar
                eng.dma_start(
                    bigt[:, sl * BW : (sl + 1) * BW].bitcast(f32),
                    edge[gsrc].bitcast(f32),
                )
        return sl

    no_we = "we" in ablate
    no_dve = "dve" in ablate

    def stage_b(g, sl):
        """Fused DoubleRow matmuls (512 cols each) + 2048-wide drains."""
        if no_we:
            return
        sv = bigv[:, sl : 4 : (3 - sl) if sl < 3 else 1, :]
        lhsT = comb[:, g * 256 : (g + 1) * 256].rearrange(
            "p (u w) -> p u w", u=2
        )
        for b in range(NBATCH):
            op = opool.tile(
                [128, 2048], f32, tag="op", name=f"r{rep}_op{g}_{b}"
            )
            c0 = b * 2048
            for q in range(4):
                nc.tensor.matmul(
                    op[:, q * 512 : (q + 1) * 512],
                    lhsT,
                    sv[:, :, c0 + q * 512 : c0 + (q + 1) * 512],
                    start=True, stop=True,
                    perf_mode=DR,
                )
            if no_dve:
                continue
            eng = _DRAIN[b]
            acc = accs[eng]
            if eng == "dve":
                if first[eng]:
                    nc.vector.tensor_copy(acc[:], op[:])
                else:
                    nc.vector.tensor_tensor(acc[:], op[:], acc[:], Alu.max)
            else:  # actd: ACT evacuates to bf16, DVE maxes from SBUF
                st = stpool.tile(
                    [128, 2048], bf16, tag="st", name=f"r{rep}_st{g}_{b}"
                )
                nc.scalar.copy(st[:], op[:])
                if first[eng]:
                    nc.vector.tensor_copy(acc[:], st[:])
                else:
                    nc.vector.tensor_tensor(acc[:], st[:], acc[:], Alu.max)
            first[eng] = False

    if "loop" not in ablate:
        depth = 2 if "pf2" in ablate else 1
        pend = []
        for g in range(NG):
            sl = stage_a(g)
            pend.append((g, sl))
            if len(pend) > depth:
                stage_b(*pend.pop(0))
        for args in pend:
            stage_b(*args)

    # ---- finalize --------------------------------------------------------
    for eng, acc in accs.items():
        if first[eng]:  # ablation runs that never touched this accumulator
            nc.vector.memset(acc[:], -3.0e38)
    r01 = const.tile([128, 2048], bf16)
    nc.vector.tensor_tensor(r01[:], accD[:], accA[:], Alu.max)
    rq = const.tile([128, 1024], bf16)
    nc.vector.tensor_tensor(rq[:], r01[:, 0:1024], r01[:, 1024:2048], Alu.max)
    rh = const.tile([128, 512], bf16)
    nc.vector.tensor_tensor(rh[:], rq[:, 0:512], rq[:, 512:1024], Alu.max)
    acc256 = const.tile([128, 256], bf16)
    nc.vector.tensor_tensor(acc256[:], rh[:, 0:256], rh[:, 256:512], Alu.max)

    ps_cv = ps_pool.tile([128, 256], f32, tag="op")
    nc.tensor.matmul(ps_cv[:], ones_1c[:], cvec[:], start=True, stop=True)
    msgsT = const.tile([128, 256], f32)
    nc.vector.tensor_tensor(msgsT[:], acc256[:], msg_nT[:], Alu.add)
    resT = const.tile([128, 256], f32r)
    nc.vector.tensor_tensor(resT[:], msgsT[:], ps_cv[:], Alu.max)

    # ret_T (o, n)
    ps_ret = ps_pool.tile([128, 256], f32, tag="op")
    nc.tensor.matmul(ps_ret[:], W_sb["Wo1"][:], nodeT[:], start=True, stop=False)
    nc.tensor.matmul(ps_ret[:], W_sb["Wo2"][:], hidT[:], start=False, stop=False)
    nc.tensor.matmul(ps_ret[:], W_sb["Wo3"][:], resT[:], start=False, stop=False)
    nc.tensor.matmul(
        ps_ret[:], B_sb["bosum"][:], ones_row[:], start=False, stop=True
    )
    retT = const.tile([128, 256], f32)
    nc.scalar.copy(retT[:], ps_ret[:])

    ps_out = ps_pool.tile([128, 256], f32, tag="op")
    for t in range(NT):
        nc.tensor.transpose(
            ps_out[:, t * 128 : (t + 1) * 128],
            retT[:, t * 128 : (t + 1) * 128],
            ident[:],
        )
    out_sb = const.tile([128, 256], f32)
    nc.scalar.copy(out_sb[:], ps_out[:])
    nc.sync.dma_start(
        out.rearrange("(t p) o -> p t o", p=128),
        out_sb[:].rearrange("p (t o) -> p t o", t=NT),
    )


def build_nc(repeat=1, edge_groups=None, loop_n=1, ablate=()):
    """Build the (single-core SPMD) Bass program; returns nc."""
    _ensure_path()
    import concourse.tile as tile
    from concourse import bacc, mybir

    f32 = mybir.dt.float32
    i32 = mybir.dt.int32
    fp8 = mybir.dt.float8e4
    bf16 = mybir.dt.bfloat16

    nc = bacc.Bacc(
        "TRN2", target_bir_lowering=False, debug=False, num_devices=NCORES
    )
    n_groups = NG if edge_groups is None else edge_groups
    WPCOLS = 7 * 128 + 2 * (N // 128) * 128 + 1
    aps = {
        "edge": nc.dram_tensor(
            "edge", [n_groups, E, GI * N], fp8, kind="ExternalInput"
        ).ap(),
        "wpack": nc.dram_tensor(
            "wpack", [128, WPCOLS], f32, kind="ExternalInput"
        ).ap(),
        "bpack": nc.dram_tensor("bpack", [2 * 128], f32, kind="ExternalInput").ap(),
        "cvec_in": nc.dram_tensor("cvec_in", [256], f32, kind="ExternalInput").ap(),
        "combstat": nc.dram_tensor(
            "combstat", [128, NG * 256], fp8, kind="ExternalInput"
        ).ap(),
        "maskrows": nc.dram_tensor(
            "maskrows", [40, GI * 256], fp8, kind="ExternalInput"
        ).ap(),
        "out": nc.dram_tensor("out", [N, OUT], f32, kind="ExternalOutput").ap(),
    }
    aps["h_scratch"] = nc.dram_tensor("h_scratch", [N * MID], fp8).ap()

    with tile.TileContext(nc) as tc:
        if loop_n > 1:
            with tc.For_i(0, loop_n, 1):
                with ExitStack() as ctx:
                    _kernel_body(
                        ctx, tc, aps, rep=0, edge_groups=edge_groups, ablate=ablate
                    )
        else:
            for rep in range(repeat):
                with ExitStack() as ctx:
                    _kernel_body(
                        ctx, tc, aps, rep=rep, edge_groups=edge_groups, ablate=ablate
                    )
    _dedup_ldweights(nc.m)
    nc.compile()
    return nc


def _get_nc():
    if "nc" not in _CACHE:
        _CACHE["nc"] = build_nc()
    return _CACHE["nc"]


def _f8(x):
    import ml_dtypes

    return np.asarray(x, np.float32).astype(ml_dtypes.float8_e4m3)


def _pack_edge(e):
    """(N, N, E) f32 -> (NG, E, GI*N) fp8 in (g, e, a, j) order."""
    x = np.asarray(e, np.float32).transpose(0, 2, 1)          # (i, e, j)
    x = x.reshape(NG, GI, E, N).transpose(0, 2, 1, 3)         # (g, e, a, j)
    return np.ascontiguousarray(_f8(x.reshape(NG, E, GI * N)))


def _maskrows_np(adj_mat):
    """[40, 8192] fp8: rows 0..31 one-hot, rows 32..39 = adjm rows of
    groups 0..7 ((adj-1)*A8 at (sender 32g+k, j) -> row 32+g col k*256+j)."""
    mr = np.zeros((40, GI * 256), np.float32)
    cols = np.arange(GI * 256)
    mr[cols // 256, cols] = 1.0
    am = (np.asarray(adj_mat, np.float32) - 1.0) * A8      # (i, j)
    mr[32:40] = am.reshape(NG, GI * 256)
    return _f8(mr)


def _combstat_np(We):
    """[128, NG*256] fp8: per group g: [We | h slots(0) rows 0..32,
    ones at row 32+g, zeros elsewhere]."""
    cs = np.zeros((128, NG * 256), np.float32)
    Wef = np.asarray(We, np.float32)
    for g in range(NG):
        cs[:, g * 256 : g * 256 + 128] = Wef
        cs[32 + g, g * 256 + 128 : (g + 1) * 256] = 1.0
    return _f8(cs)


def _nat_tiles(x):
    return np.ascontiguousarray(
        np.asarray(x, np.float32)
        .reshape(NT, 128, 128)
        .transpose(1, 0, 2)
        .reshape(128, NT * 128)
    )


def make_in_maps(**inputs):
    """Shard full inputs into per-core input maps (batch-parallel)."""
    cs = _combstat_np(inputs["We"])
    wcommon = np.concatenate(
        [np.ascontiguousarray(inputs[w], np.float32) for w in _WNAMES], axis=1
    )
    bsum = (np.asarray(inputs["bn"]) + inputs["bh"] + inputs["be"]
            + inputs["bg"]).astype(np.float32)
    bosum = (np.asarray(inputs["bo1"]) + inputs["bo2"]
             + inputs["bo3"]).astype(np.float32)
    bpack = np.concatenate([bsum, bosum])
    in_maps = []
    for c in range(NCORES):
        wpack = np.concatenate(
            [
                wcommon,
                _nat_tiles(inputs["node_fts"][c]),
                _nat_tiles(inputs["hidden"][c]),
                np.asarray(inputs["graph_fts"][c], np.float32).reshape(128, 1),
            ],
            axis=1,
        )
        am = np.asarray(inputs["adj_mat"][c], np.float32)
        cvec_in = np.where(am.min(axis=0) > 0.5, -BIG, 0.0).astype(np.float32)
        m = {
            "edge": _pack_edge(inputs["edge_fts"][c]),
            "wpack": np.ascontiguousarray(wpack, np.float32),
            "bpack": bpack,
            "cvec_in": cvec_in,
            "combstat": cs,
            "maskrows": _maskrows_np(inputs["adj_mat"][c]),
        }
        in_maps.append(m)
    return in_maps


def kernel(**inputs) -> np.ndarray:
    """Full-input entry point: shards over 8 cores, returns (B, N, OUT)."""
    _ensure_path()
    from concourse import bass_utils

    nc = _get_nc()
    in_maps = make_in_maps(**inputs)
    res = bass_utils.run_bass_kernel_spmd(nc, in_maps, core_ids=list(range(NCORES)))
    outs = [res.results[c]["out"] for c in range(NCORES)]
    return np.stack(outs, axis=0).astype(np.float32)


if __name__ == "__main__":
    rng = np.random.default_rng(0)
    inputs = {
        "node_fts": rng.normal(size=(B, N, D)).astype(np.float32),
        "edge_fts": rng.normal(size=(B, N, N, E)).astype(np.float32),
        "graph_fts": rng.normal(size=(B, G)).astype(np.float32),
        "adj_mat": rng.integers(0, 2, size=(B, N, N)).astype(np.int32),
        "hidden": rng.normal(size=(B, N, D)).astype(np.float32),
    }
    s = 0.02
    for w in _WNAMES:
        inputs[w] = (s * rng.normal(size=(128, 128))).astype(np.float32)
    for b in _BNAMES:
        inputs[b] = np.zeros(128, np.float32)
    out = kernel(**inputs)
    print(out.shape, out.dtype)
